# revision 26
# baseline (speedup 1.0000x reference)
"""Multi-head attention block (B=8, N=1024, D=1024, H=16, dh=64) on 8 TRN2 NeuronCores.

Strategy: data-parallel over batch (1 batch element per core). Per core, the whole
attention block runs out of SBUF in a feature-major ("transposed") dataflow that
avoids all on-device transposes:

  - qT/kT computed feature-major:  qkT[j, n]  = sum_d qkv_w[j, d] * x[n, d]   (lhsT=Wqk^T, rhs=x^T)
  - v computed token-major:        v[n, j]    = sum_d x[n, d] * Wv[j, d]      (lhsT=x^T, rhs=Wv^T)
  - scores transposed:             sT[k, q]   = sum_dh kT[dh, k] * qT[dh, q]  (K=64, row-packed head pairs)
  - pattern:                       pT = exp(SCALE * sT)                        (ACT, PSUM->SBUF, fp16)
  - zT + denominator fused:        [zT_h; den] = [v_h | 1]^T @ pT              (M=65, ones col)
  - normalize:                     zT_h *= bcast(1/den)   (DVE recip -> GPSIMD partition_broadcast -> DVE mul)
  - output transposed:             outT[c, q] = sum_j proj_w[c, j] zT[j, q] + pb[c]

All matmul operands live in SBUF as fp16 (same 1 cyc/row PE rate as bf16/f32r
with free>=256, but half the DMA/SBUF footprint); accumulation is fp32 PSUM.
The V projection runs kt-outer in 2-token-tile waves so the first matmul only
needs the first x/wv DMA chunks, hiding the input load almost entirely.
"""
import os
import numpy as np
from contextlib import ExitStack

import concourse.bacc as bacc
import concourse.tile as tile
from concourse import mybir
from concourse.bass_utils import run_bass_kernel_spmd

f32 = mybir.dt.float32
f32r = mybir.dt.float32r
f16 = mybir.dt.float16
AF = mybir.ActivationFunctionType

NB = 8          # batch / cores
N = 1024        # tokens
D = 1024        # d_model
H = 16          # heads
DH = 64         # head dim
SCALE = DH ** -0.5
NT = N // 128   # 8 token tiles
DT = D // 128   # 8 d tiles
HP = H // 2     # 8 head pairs

# Stashed results of the last run (for test harness introspection)
LAST_RESULTS = None
_NC_CACHE = None


def build_nc(loop_r=None):
    nc = bacc.Bacc("TRN2", target_bir_lowering=False, debug=False, enable_asserts=False)

    xp = nc.dram_tensor("xp", [128, DT * N], f16, kind="ExternalInput").ap()
    wqk = nc.dram_tensor("wqk", [128, HP * 2048], f16, kind="ExternalInput").ap()
    wv = nc.dram_tensor("wv", [128, 8192], f16, kind="ExternalInput").ap()
    pw = nc.dram_tensor("pw", [128, 8192], f16, kind="ExternalInput").ap()
    bqk = nc.dram_tensor("bqk", [128, 16], f32, kind="ExternalInput").ap()
    bv = nc.dram_tensor("bv", [1, 1024], f16, kind="ExternalInput").ap()
    pb = nc.dram_tensor("pb", [128, 8], f32, kind="ExternalInput").ap()
    outT = nc.dram_tensor("outT", [D, N], f32, kind="ExternalOutput").ap()

    with tile.TileContext(nc) as tc, ExitStack() as ctx:
        const = ctx.enter_context(tc.tile_pool(name="const", bufs=1))
        xpool = ctx.enter_context(tc.tile_pool(name="xp", bufs=1))
        wvpool = ctx.enter_context(tc.tile_pool(name="wvp", bufs=1))
        vpool = ctx.enter_context(tc.tile_pool(name="vp", bufs=1))
        qkpool = ctx.enter_context(tc.tile_pool(name="qkp", bufs=4))
        ztpool = ctx.enter_context(tc.tile_pool(name="ztp", bufs=1))
        wqkpool = ctx.enter_context(tc.tile_pool(name="wqkp", bufs=2))
        pwpool = ctx.enter_context(tc.tile_pool(name="pwp", bufs=1))
        ptpool = ctx.enter_context(tc.tile_pool(name="ptp", bufs=4))
        mpool = ctx.enter_context(tc.tile_pool(name="mp", bufs=2))
        psum = ctx.enter_context(tc.tile_pool(name="ps", bufs=1, space="PSUM"))

        if loop_r is not None:
            ctx.enter_context(tc.For_i(
                0, loop_r, 1,
                hint_engines=(mybir.EngineType.PE, mybir.EngineType.Activation,
                              mybir.EngineType.DVE, mybir.EngineType.SP,
                              mybir.EngineType.Pool),
            ))

        # ---- persistent activations. Three HWDGE queues run in parallel:
        # wv chunks on ACT (idle until attention), xT pieces on SP (strided,
        # one issue per piece class; V runs kt-outer so wave w only needs
        # xT cols [256w, 256w+256) of each chunk), consts on DVE. ----
        bv_row = const.tile([1, 1024], f16, tag="bvr")
        bvrep = const.tile([128, 1024], f16, tag="bvrep")

        xT = xpool.tile([128, DT * N], f16, tag="xT")        # [p, kt*N + n] = x[n, 128kt+p]
        wv_sb = wvpool.tile([128, 8192], f16, tag="wv")      # [p, kt*1024 + j]
        x3 = xT.rearrange("p (kt n) -> p kt n", n=N)
        s3 = xp.rearrange("p (kt n) -> p kt n", n=N)
        for g4 in range(4):
            nc.scalar.dma_start(wv_sb[:, g4 * 2048: (g4 + 1) * 2048],
                                wv[:, g4 * 2048: (g4 + 1) * 2048])
            nc.sync.dma_start(x3[:, 2 * g4: 2 * g4 + 2, 0:256],
                              s3[:, 2 * g4: 2 * g4 + 2, 0:256])
            if g4 == 1:
                nc.sync.dma_start(bv_row, bv)
                nc.gpsimd.partition_broadcast(bvrep, bv_row)
        nc.sync.dma_start(x3[:, 0:4, 256:512], s3[:, 0:4, 256:512])
        nc.sync.dma_start(x3[:, 4:8, 256:512], s3[:, 4:8, 256:512])
        nc.sync.dma_start(x3[:, :, 512:1024], s3[:, :, 512:1024])

        bqk_sb = const.tile([128, 16], f32, tag="bqk")
        nc.scalar.dma_start(bqk_sb, bqk)
        pb_sb = const.tile([128, 8], f32, tag="pb")
        nc.scalar.dma_start(pb_sb, pb)

        v_sb = []                                            # [p=token, 65h + c]; col 65h+64 == 1.0
        for tt in range(NT):
            vt = vpool.tile([128, H * 65], f16, tag=f"v{tt}", name=f"v{tt}")
            nc.gpsimd.memset(vt.rearrange("p (h c) -> p h c", c=65)[:, :, 64], 1.0)
            v_sb.append(vt)

        zt = []                                              # [p=feature within tile, q]
        for jt in range(DT):
            zt.append(ztpool.tile([128, N], f16, tag=f"z{jt}", name=f"z{jt}"))

        # ---- interleaved qkT projection + attention ----
        def make_qk_proj(hp):
            """Returns (qa, ka, generator). Generator emits 2 PE matmuls per step,
            16 steps total, with the DVE bias-evacuation attached to group ends."""
            wqk_t = wqkpool.tile([128, 2048], f16, tag="wqk", name=f"wqk{hp}")
            nc.sync.dma_start(wqk_t, wqk[:, hp * 2048: (hp + 1) * 2048])
            qa = qkpool.tile([128, N], f16, tag="qk", name=f"qa{hp}")
            ka = qkpool.tile([128, N], f16, tag="qk", name=f"ka{hp}")

            def gen():
                for dest, jt, which in ((qa, hp, 0), (ka, 8 + hp, 1)):
                    for qn in range(2):
                        ps = psum.tile([128, 512], f32, tag="mm", name="ps_qk", bufs=1)
                        for kt in range(DT):
                            base = kt * 256 + which * 128
                            nc.tensor.matmul(
                                ps,
                                wqk_t[:, base: base + 128],
                                xT[:, kt * N + qn * 512: kt * N + qn * 512 + 512],
                                start=(kt == 0), stop=(kt == DT - 1),
                            )
                            if kt % 2 == 1:
                                if kt == DT - 1:
                                    with nc.allow_low_precision(reason="f16 qk evac"):
                                        nc.vector.tensor_scalar_add(
                                            dest[:, qn * 512: qn * 512 + 512], ps,
                                            bqk_sb[:, jt: jt + 1],
                                        )
                                yield
            return qa, ka, gen()

        # ---- phase V: v projection, kt-outer 2-tile waves. PSUM slot-pairs
        # rotate over A,B (big), C (zps0+zps1), D (zps2+mm) so a wave never
        # waits on the previous wave's evacuation; the hp0 qk projection is
        # interleaved into the last wave out of the then-free mm slot. ----
        qa, ka, g = make_qk_proj(0)

        def v_slot(kind):
            if kind == "A" or kind == "B":
                t = psum.tile([128, 1024], f32, tag="big", name=f"vps{kind}", bufs=2)
                return [t[:, 0:512], t[:, 512:1024]]
            if kind == "C":
                return [psum.tile([128, 512], f32, tag="zps", name="vpsc0", bufs=3),
                        psum.tile([128, 512], f32, tag="zps", name="vpsc1", bufs=3)]
            return [psum.tile([128, 512], f32, tag="zps", name="vpsd0", bufs=3),
                    psum.tile([128, 512], f32, tag="mm", name="vpsd1", bufs=1)]

        wave_slots = [("A", "B"), ("C", "D"), ("A", "B"), ("C", "A")]
        for w in range(NT // 2):
            vps = [v_slot(k) for k in wave_slots[w]]
            for kt in range(DT):
                for i in range(2):
                    tt = 2 * w + i
                    for jn in range(2):
                        nc.tensor.matmul(
                            vps[i][jn],
                            xT[:, kt * N + tt * 128: kt * N + tt * 128 + 128],
                            wv_sb[:, kt * 1024 + jn * 512: kt * 1024 + jn * 512 + 512],
                            start=(kt == 0), stop=(kt == DT - 1),
                        )
                if w == NT // 2 - 1 and kt >= 2:
                    next(g, None)
                    next(g, None)
                    next(g, None)
            for i in range(2):
                tt = 2 * w + i
                v3 = v_sb[tt].rearrange("p (h c) -> p h c", c=65)
                b3 = bvrep.rearrange("p (h c) -> p h c", c=64)
                for jn in range(2):
                    with nc.allow_low_precision(reason="f16 v evac"):
                        nc.vector.tensor_add(
                            v3[:, jn * 8: jn * 8 + 8, 0:64],
                            vps[i][jn].rearrange("p (h c) -> p h c", c=64),
                            b3[:, jn * 8: jn * 8 + 8, :],
                        )
        for _ in g:  # drain leftovers
            pass

        # whole proj weight, preloaded during attention (see hp loop)
        pw_sb = pwpool.tile([128, 8192], f16, tag="pw")

        def make_proj_prefix():
            """Filler for the last head-pair: proj (ct0, qn0/qn1) partials
            jt 0..6 accumulate in the freed mm / third zps slots while hp7's
            attention runs."""
            ps_pre = [psum.tile([128, 512], f32, tag="mm", name="ps_o0", bufs=1), None]

            def gen():
                for jt in range(DT - 1):
                    nc.tensor.matmul(
                        ps_pre[0], pw_sb[:, jt * 128: jt * 128 + 128],
                        zt[jt][:, 0:512], start=(jt == 0), stop=False,
                    )
                    yield
                # allocated mid-qn0 so the zps rotation skips hp7's live slots
                ps_pre[1] = psum.tile([128, 512], f32, tag="zps", name="ps_o1", bufs=3)
                for jt in range(DT - 1):
                    nc.tensor.matmul(
                        ps_pre[1], pw_sb[:, jt * 128: jt * 128 + 128],
                        zt[jt][:, 512:1024], start=(jt == 0), stop=False,
                    )
                    yield
            return ps_pre, gen()

        def attention(hp, qa, ka, filler):
            def emit_sps_exp(qn, kt):
                sps = psum.tile([128, 1024], f32, tag="big", name="sps", bufs=2)
                for h in range(2):
                    off = h * 64
                    nc.tensor.matmul(
                        sps[:, h * 512: h * 512 + 512],
                        ka[off: off + 64, kt * 128: kt * 128 + 128],
                        qa[off: off + 64, qn * 512: qn * 512 + 512],
                        start=True, stop=True,
                    )
                pt = ptpool.tile([128, 1024], f16, tag="pt", name="pt")
                nc.scalar.activation(pt, sps, AF.Exp, scale=SCALE)
                return pt

            for qn in range(2):
                zps = [psum.tile([65, 512], f32, tag="zps", name=f"zps{h}", bufs=3)
                       for h in range(2)]
                pt_next = emit_sps_exp(qn, 0)
                for kt in range(NT):
                    pt = pt_next
                    if kt + 1 < NT:
                        pt_next = emit_sps_exp(qn, kt + 1)
                    if filler is not None:
                        next(filler, None)
                    for h in range(2):
                        nc.tensor.matmul(
                            zps[h],
                            v_sb[kt][:, 65 * (2 * hp + h): 65 * (2 * hp + h) + 65],
                            pt[:, h * 512: h * 512 + 512],
                            start=(kt == 0), stop=(kt == NT - 1),
                        )
                for h in range(2):
                    recip = mpool.tile([1, 512], f32r, tag="recip", name="recip")
                    with nc.allow_low_precision(reason="f32r rounding of softmax denom"):
                        nc.vector.reciprocal(recip, zps[h][64:65, :])
                    bc = mpool.tile([64, 512], f32r, tag="bc", name="bc", bufs=4)
                    nc.gpsimd.partition_broadcast(bc, recip)
                    with nc.allow_low_precision(reason="f16 attn out"):
                        nc.vector.tensor_mul(
                            zt[hp][h * 64: h * 64 + 64, qn * 512: qn * 512 + 512],
                            zps[h][0:64, :], bc,
                        )

        ps_pre = None
        for hp in range(HP):
            if hp == 4:
                # prefetch proj weights while the DMA queues are idle
                nc.sync.dma_start(pw_sb[:, 0:4096], pw[:, 0:4096])
                nc.sync.dma_start(pw_sb[:, 4096:8192], pw[:, 4096:8192])
            if hp + 1 < HP:
                nqa, nka, ng = make_qk_proj(hp + 1)
            else:
                nqa = nka = None
                ps_pre, ng = make_proj_prefix()
            attention(hp, qa, ka, ng)
            if ng is not None:
                for _ in ng:  # drain leftovers
                    pass
            qa, ka = nqa, nka

        # ---- output projection (transposed) ----
        # NOTE: must be emitted entirely AFTER the attention loop: Tile
        # dependencies follow emission order, so reads of zt must come after
        # all writes.
        # group order: ct0-qn0 finisher first, ct0-qn1 finisher deferred past
        # ct1 (it waits on hp7-qn1's normalize chain) and emitted LAST so the
        # tail chain is a single short matmul+evac+DMA.
        def emit_group(ct, qn):
            pw_t = pw_sb[:, ct * 1024: (ct + 1) * 1024]
            if ct == 0:
                # finish the prefix accumulation started during hp7
                ps = ps_pre[qn]
                nc.tensor.matmul(
                    ps, pw_t[:, (DT - 1) * 128: DT * 128],
                    zt[DT - 1][:, qn * 512: qn * 512 + 512], start=False, stop=True,
                )
            else:
                ps = psum.tile([128, 512], f32, tag="big", name="ps_o", bufs=2)
                for jt in range(DT):
                    nc.tensor.matmul(
                        ps,
                        pw_t[:, jt * 128: jt * 128 + 128],
                        zt[jt][:, qn * 512: qn * 512 + 512],
                        start=(jt == 0), stop=(jt == DT - 1),
                    )
            ot = mpool.tile([128, 512], f32, tag="ot", name="ot", bufs=4)
            nc.scalar.activation(ot, ps, AF.Identity, bias=pb_sb[:, ct: ct + 1])
            nc.sync.dma_start(outT[ct * 128: ct * 128 + 128, qn * 512: qn * 512 + 512], ot)

        order = [(0, 0)]
        for ct in range(1, DT):
            order += [(ct, 0), (ct, 1)]
        order.append((0, 1))
        for ct, qn in order:
            emit_group(ct, qn)

    nc.compile()
    return nc


def prep_inputs(x, qkv_w, qkv_b, proj_w, proj_b):
    x = np.asarray(x, dtype=np.float32)
    qkv_w = np.asarray(qkv_w, dtype=np.float32)
    qkv_b = np.asarray(qkv_b, dtype=np.float32)
    proj_w = np.asarray(proj_w, dtype=np.float32)
    proj_b = np.asarray(proj_b, dtype=np.float32)

    # x^T packed: [b, p, kt*N + n] = x[b, n, 128kt+p]
    xp = (x.transpose(0, 2, 1).reshape(NB, DT, 128, N).transpose(0, 2, 1, 3)
          .reshape(NB, 128, DT * N).astype(np.float16))

    wqkT = qkv_w[:2048, :].T                                  # [d, j']
    A4 = wqkT.reshape(DT, 128, 16, 128).transpose(1, 0, 2, 3)  # [p, kt, jt, jj]
    wqk_packed = (np.stack([A4[:, :, 0:8, :], A4[:, :, 8:16, :]], axis=3)
                  .transpose(0, 2, 1, 3, 4).reshape(128, HP * 2048).astype(np.float16))

    wvT = qkv_w[2048:, :].T                                   # [d, j]
    wv_packed = (wvT.reshape(DT, 128, 1024).transpose(1, 0, 2)
                 .reshape(128, 8192).astype(np.float16))

    pwT = proj_w.T                                            # [j, c]
    pw_packed = (pwT.reshape(DT, 128, DT, 128).transpose(1, 2, 0, 3)
                 .reshape(128, 8192).astype(np.float16))

    bqk_pt = np.ascontiguousarray(qkv_b[:2048].reshape(16, 128).T)
    bv_r = qkv_b[2048:].reshape(1, 1024).astype(np.float16)
    pb_pt = np.ascontiguousarray(proj_b.reshape(8, 128).T)

    shared = {
        "wqk": wqk_packed, "wv": wv_packed, "pw": pw_packed,
        "bqk": bqk_pt, "bv": bv_r, "pb": pb_pt,
    }
    return [{**shared, "xp": xp[b]} for b in range(NB)]


def kernel(x, qkv_w, qkv_b, proj_w, proj_b):
    global LAST_RESULTS, _NC_CACHE
    if _NC_CACHE is None:
        _NC_CACHE = build_nc()
    nc = _NC_CACHE
    in_maps = prep_inputs(x, qkv_w, qkv_b, proj_w, proj_b)
    res = run_bass_kernel_spmd(
        nc, in_maps, core_ids=list(range(NB)),
        trace=bool(os.environ.get("BASS_TRACE")),
    )
    LAST_RESULTS = res
    out = np.stack([np.ascontiguousarray(res.results[b]["outT"].T) for b in range(NB)])
    return out


# revision 44
# speedup vs baseline: 1.8399x; 1.8399x over previous
"""Multi-head attention block (B=8, N=1024, D=1024, H=16, dh=64) on 8 TRN2 NeuronCores.

Strategy: data-parallel over batch (1 batch element per core). Per core, the whole
attention block runs out of SBUF in a feature-major ("transposed") dataflow that
avoids all on-device transposes:

  - qT/kT computed feature-major:  qkT[j, n]  = sum_d qkv_w[j, d] * x[n, d]   (lhsT=Wqk^T, rhs=x^T)
  - v computed token-major:        v[n, j]    = sum_d x[n, d] * Wv[j, d]      (lhsT=x^T, rhs=Wv^T)
  - scores transposed:             sT[k, q]   = sum_dh kT[dh, k] * qT[dh, q]  (K=64, row-packed head pairs)
  - pattern:                       pT = exp(SCALE * sT)                        (ACT, PSUM->SBUF, fp16)
  - zT + denominator fused:        [zT_h; den] = [v_h | 1]^T @ pT              (M=65, ones col)
  - normalize:                     zT_h *= bcast(1/den)   (DVE recip -> GPSIMD partition_broadcast -> DVE mul)
  - output transposed:             outT[c, q] = sum_j proj_w[c, j] zT[j, q] + pb[c]

All matmul operands live in SBUF as fp16 (same 1 cyc/row PE rate as bf16/f32r
with free>=256, but half the DMA/SBUF footprint); accumulation is fp32 PSUM.

Pipeline structure (all engines near-continuously busy):
  - V projection runs kt-outer in 2-token-tile waves over 4 rotating PSUM
    slot-pairs, so the first matmul needs only the first x/wv DMA chunks and
    waves never wait on evacuations; the hp0 qk projection interleaves into
    the last wave.
  - Per head-pair, the next pair's qk projection interleaves into the
    attention kt loop; the last pair interleaves the first output-projection
    partials instead.
  - Softmax normalization is deferred off the critical path: z/den rows are
    copied to SBUF right away (freeing PSUM), and each head-pair's
    recip/broadcast/multiply burst is flushed at the START of the next
    pair's attention (one GPSIMD broadcast launch per head-pair). This is
    worth ~100us/iter on HW: the ucode launch + cross-engine semaphore chain
    otherwise gates the PSUM rotation at every qn boundary.
  - proj weights prefetch during attention; the ct0 groups finish partials
    started during hp7, with the qn1 finisher emitted last (it waits on
    hp7's normalize) and split for a short tail.
"""
import os
import numpy as np
from contextlib import ExitStack

import concourse.bacc as bacc
import concourse.tile as tile
from concourse import mybir
from concourse.bass_utils import run_bass_kernel_spmd

f32 = mybir.dt.float32
f32r = mybir.dt.float32r
f16 = mybir.dt.float16
AF = mybir.ActivationFunctionType

NB = 8          # batch / cores
N = 1024        # tokens
D = 1024        # d_model
H = 16          # heads
DH = 64         # head dim
SCALE = DH ** -0.5
NT = N // 128   # 8 token tiles
DT = D // 128   # 8 d tiles
HP = H // 2     # 8 head pairs

# Stashed results of the last run (for test harness introspection)
LAST_RESULTS = None
_NC_CACHE = None


def build_nc(loop_r=None):
    nc = bacc.Bacc("TRN2", target_bir_lowering=False, debug=False, enable_asserts=False)

    xp = nc.dram_tensor("xp", [128, DT * N], f16, kind="ExternalInput").ap()
    wqk = nc.dram_tensor("wqk", [128, HP * 2048], f16, kind="ExternalInput").ap()
    wv = nc.dram_tensor("wv", [128, 8192], f16, kind="ExternalInput").ap()
    pw = nc.dram_tensor("pw", [128, 8192], f16, kind="ExternalInput").ap()
    bqk = nc.dram_tensor("bqk", [128, 16], f32, kind="ExternalInput").ap()
    bv = nc.dram_tensor("bv", [1, 1024], f16, kind="ExternalInput").ap()
    pb = nc.dram_tensor("pb", [128, 8], f32, kind="ExternalInput").ap()
    outT = nc.dram_tensor("outT", [D, N], f32, kind="ExternalOutput").ap()

    with tile.TileContext(nc) as tc, ExitStack() as ctx:
        const = ctx.enter_context(tc.tile_pool(name="const", bufs=1))
        xpool = ctx.enter_context(tc.tile_pool(name="xp", bufs=1))
        wvpool = ctx.enter_context(tc.tile_pool(name="wvp", bufs=1))
        vpool = ctx.enter_context(tc.tile_pool(name="vp", bufs=1))
        qkpool = ctx.enter_context(tc.tile_pool(name="qkp", bufs=4))
        ztpool = ctx.enter_context(tc.tile_pool(name="ztp", bufs=1))
        wqkpool = ctx.enter_context(tc.tile_pool(name="wqkp", bufs=2))
        pwpool = ctx.enter_context(tc.tile_pool(name="pwp", bufs=1))
        ptpool = ctx.enter_context(tc.tile_pool(name="ptp", bufs=4))
        mpool = ctx.enter_context(tc.tile_pool(name="mp", bufs=2))
        psum = ctx.enter_context(tc.tile_pool(name="ps", bufs=1, space="PSUM"))

        if loop_r is not None:
            ctx.enter_context(tc.For_i(
                0, loop_r, 1,
                hint_engines=(mybir.EngineType.PE, mybir.EngineType.Activation,
                              mybir.EngineType.DVE, mybir.EngineType.SP,
                              mybir.EngineType.Pool),
            ))

        # ---- persistent activations. Three HWDGE queues run in parallel:
        # wv chunks on ACT (idle until attention), xT pieces on SP (strided,
        # one issue per piece class; V runs kt-outer so wave w only needs
        # xT cols [256w, 256w+256) of each chunk), consts on DVE. ----
        bv_row = const.tile([1, 1024], f16, tag="bvr")
        bvrep = const.tile([128, 1024], f16, tag="bvrep")

        xT = xpool.tile([128, DT * N], f16, tag="xT")        # [p, kt*N + n] = x[n, 128kt+p]
        wv_sb = wvpool.tile([128, 8192], f16, tag="wv")      # [p, kt*1024 + j]
        x3 = xT.rearrange("p (kt n) -> p kt n", n=N)
        s3 = xp.rearrange("p (kt n) -> p kt n", n=N)
        nc.scalar.dma_start(wv_sb[:, 0:1024], wv[:, 0:1024])
        for g4 in range(4):
            if g4 > 0:
                nc.scalar.dma_start(wv_sb[:, g4 * 2048: (g4 + 1) * 2048],
                                    wv[:, g4 * 2048: (g4 + 1) * 2048])
            nc.sync.dma_start(xT[:, 2 * g4 * N: (2 * g4 + 2) * N],
                              xp[:, 2 * g4 * N: (2 * g4 + 2) * N])
            if g4 == 0:
                nc.scalar.dma_start(wv_sb[:, 1024:2048], wv[:, 1024:2048])
            if g4 == 1:
                nc.sync.dma_start(bv_row, bv)
                nc.gpsimd.partition_broadcast(bvrep, bv_row)

        bqk_sb = const.tile([128, 16], f32, tag="bqk")
        nc.scalar.dma_start(bqk_sb, bqk)
        pb_sb = const.tile([128, 8], f32, tag="pb")
        nc.scalar.dma_start(pb_sb, pb)

        v_sb = []                                            # [p=token, 65h + c]; col 65h+64 == 1.0
        for tt in range(NT):
            vt = vpool.tile([128, H * 65], f16, tag=f"v{tt}", name=f"v{tt}")
            nc.gpsimd.memset(vt.rearrange("p (h c) -> p h c", c=65)[:, :, 64], 1.0)
            v_sb.append(vt)

        zt = []                                              # [p=feature within tile, q]
        for jt in range(DT):
            zt.append(ztpool.tile([128, N], f16, tag=f"z{jt}", name=f"z{jt}"))

        # ---- interleaved qkT projection + attention ----
        def make_qk_proj(hp):
            """Returns (qa, ka, generator). Generator emits 2 PE matmuls per step,
            16 steps total, with the DVE bias-evacuation attached to group ends."""
            wqk_t = wqkpool.tile([128, 2048], f16, tag="wqk", name=f"wqk{hp}")
            nc.sync.dma_start(wqk_t, wqk[:, hp * 2048: (hp + 1) * 2048])
            qa = qkpool.tile([128, N], f16, tag="qk", name=f"qa{hp}")
            ka = qkpool.tile([128, N], f16, tag="qk", name=f"ka{hp}")

            def gen():
                for dest, jt, which in ((qa, hp, 0), (ka, 8 + hp, 1)):
                    for qn in range(2):
                        ps = psum.tile([128, 512], f32, tag="mm", name="ps_qk", bufs=1)
                        for kt in range(DT):
                            base = kt * 256 + which * 128
                            nc.tensor.matmul(
                                ps,
                                wqk_t[:, base: base + 128],
                                xT[:, kt * N + qn * 512: kt * N + qn * 512 + 512],
                                start=(kt == 0), stop=(kt == DT - 1),
                            )
                            if kt % 2 == 1:
                                if kt == DT - 1:
                                    with nc.allow_low_precision(reason="f16 qk evac"):
                                        nc.vector.tensor_scalar_add(
                                            dest[:, qn * 512: qn * 512 + 512], ps,
                                            bqk_sb[:, jt: jt + 1],
                                        )
                                yield
            return qa, ka, gen()

        # ---- phase V: v projection, kt-outer 2-tile waves. PSUM slot-pairs
        # rotate over A,B (big), C (zps0+zps1), D (zps2+mm) so a wave never
        # waits on the previous wave's evacuation; the hp0 qk projection is
        # interleaved into the last wave out of the then-free mm slot. ----
        qa, ka, g = make_qk_proj(0)

        def v_slot(kind):
            if kind == "A" or kind == "B":
                t = psum.tile([128, 1024], f32, tag="big", name=f"vps{kind}", bufs=2)
                return [t[:, 0:512], t[:, 512:1024]]
            if kind == "C":
                return [psum.tile([128, 512], f32, tag="zps", name="vpsc0", bufs=3),
                        psum.tile([128, 512], f32, tag="zps", name="vpsc1", bufs=3)]
            return [psum.tile([128, 512], f32, tag="zps", name="vpsd0", bufs=3),
                    psum.tile([128, 512], f32, tag="mm", name="vpsd1", bufs=1)]

        wave_slots = [("A", "B"), ("C", "D"), ("A", "B"), ("C", "A")]
        for w in range(NT // 2):
            vps = [v_slot(k) for k in wave_slots[w]]
            for kt in range(DT):
                for i in range(2):
                    tt = 2 * w + i
                    for jn in range(2):
                        nc.tensor.matmul(
                            vps[i][jn],
                            xT[:, kt * N + tt * 128: kt * N + tt * 128 + 128],
                            wv_sb[:, kt * 1024 + jn * 512: kt * 1024 + jn * 512 + 512],
                            start=(kt == 0), stop=(kt == DT - 1),
                        )
                if w == NT // 2 - 1 and kt >= 2:
                    next(g, None)
                    next(g, None)
                    next(g, None)
            for i in range(2):
                tt = 2 * w + i
                v3 = v_sb[tt].rearrange("p (h c) -> p h c", c=65)
                b3 = bvrep.rearrange("p (h c) -> p h c", c=64)
                for jn in range(2):
                    with nc.allow_low_precision(reason="f16 v evac"):
                        nc.vector.tensor_add(
                            v3[:, jn * 8: jn * 8 + 8, 0:64],
                            vps[i][jn].rearrange("p (h c) -> p h c", c=64),
                            b3[:, jn * 8: jn * 8 + 8, :],
                        )
        for _ in g:  # drain leftovers
            pass

        # whole proj weight, preloaded during attention (see hp loop)
        pw_sb = pwpool.tile([128, 8192], f16, tag="pw")

        def make_proj_prefix():
            """Filler for the last head-pair: proj (ct0, qn0/qn1) partials
            jt 0..6 accumulate in the freed mm / third zps slots while hp7's
            attention runs."""
            ps_pre = [psum.tile([128, 512], f32, tag="mm", name="ps_o0", bufs=1), None]

            def gen():
                for jt in range(DT - 1):
                    nc.tensor.matmul(
                        ps_pre[0], pw_sb[:, jt * 128: jt * 128 + 128],
                        zt[jt][:, 0:512], start=(jt == 0), stop=False,
                    )
                    yield
                # allocated mid-qn0 so the zps rotation skips hp7's live slots
                ps_pre[1] = psum.tile([128, 512], f32, tag="zps", name="ps_o1", bufs=3)
                for jt in range(DT - 1):
                    nc.tensor.matmul(
                        ps_pre[1], pw_sb[:, jt * 128: jt * 128 + 128],
                        zt[jt][:, 512:1024], start=(jt == 0), stop=False,
                    )
                    yield
            return ps_pre, gen()

        # deferred softmax-normalize: both qn of a head-pair flushed in one
        # burst (one broadcast ucode launch per hp) after the NEXT head-pair's
        # first z-evacuation, keeping the DVE queue clear at qn boundaries
        norm_pending = []

        def flush_norm(entries):
            recip = mpool.tile([1, 2048], f32r, tag="recip", name="recip", bufs=2)
            for i, (hp, qn, zsb) in enumerate(entries):
                for h in range(2):
                    with nc.allow_low_precision(reason="f32r rounding of softmax denom"):
                        nc.vector.reciprocal(
                            recip[:, (2 * i + h) * 512: (2 * i + h) * 512 + 512],
                            zsb[h][64:65, :])
            bc = mpool.tile([64, 2048], f32r, tag="bc", name="bc", bufs=2)
            nc.gpsimd.partition_broadcast(bc, recip)
            for i, (hp, qn, zsb) in enumerate(entries):
                for h in range(2):
                    with nc.allow_low_precision(reason="f16 attn out"):
                        nc.vector.tensor_mul(
                            zt[hp][h * 64: h * 64 + 64, qn * 512: qn * 512 + 512],
                            zsb[h][0:64, :],
                            bc[:, (2 * i + h) * 512: (2 * i + h) * 512 + 512])

        def attention(hp, qa, ka, filler):
            # flush the previous head-pair's deferred normalize first: its
            # zt writes must precede any filler reads of that zt tile
            if norm_pending:
                flush_norm(norm_pending[:])
                del norm_pending[:]

            def emit_sps_exp(qn, kt):
                sps = psum.tile([128, 1024], f32, tag="big", name="sps", bufs=2)
                for h in range(2):
                    off = h * 64
                    nc.tensor.matmul(
                        sps[:, h * 512: h * 512 + 512],
                        ka[off: off + 64, kt * 128: kt * 128 + 128],
                        qa[off: off + 64, qn * 512: qn * 512 + 512],
                        start=True, stop=True,
                    )
                pt = ptpool.tile([128, 1024], f16, tag="pt", name="pt")
                nc.scalar.activation(pt, sps, AF.Exp, scale=SCALE)
                return pt

            for qn in range(2):
                zps = [psum.tile([65, 512], f32, tag="zps", name=f"zps{h}", bufs=3)
                       for h in range(2)]
                pt_next = emit_sps_exp(qn, 0)
                for kt in range(NT):
                    pt = pt_next
                    if kt + 1 < NT:
                        pt_next = emit_sps_exp(qn, kt + 1)
                    if filler is not None:
                        next(filler, None)
                    for h in range(2):
                        nc.tensor.matmul(
                            zps[h],
                            v_sb[kt][:, 65 * (2 * hp + h): 65 * (2 * hp + h) + 65],
                            pt[:, h * 512: h * 512 + 512],
                            start=(kt == 0), stop=(kt == NT - 1),
                        )
                # evacuate zps to SBUF immediately (frees the PSUM slot for
                # the next qn); the actual normalize runs later, off the
                # critical path (see flush_norm)
                zsb = [mpool.tile([65, 512], f32r, tag="zsb", name=f"zsb{h}", bufs=8)
                       for h in range(2)]
                for h in range(2):
                    with nc.allow_low_precision(reason="f32r z evac"):
                        nc.vector.tensor_copy(zsb[h], zps[h])
                norm_pending.append((hp, qn, zsb))

        ps_pre = None
        for hp in range(HP):
            if hp == 4:
                # prefetch proj weights while the DMA queues are idle
                nc.sync.dma_start(pw_sb[:, 0:4096], pw[:, 0:4096])
                nc.sync.dma_start(pw_sb[:, 4096:8192], pw[:, 4096:8192])
            if hp + 1 < HP:
                nqa, nka, ng = make_qk_proj(hp + 1)
            else:
                nqa = nka = None
                ps_pre, ng = make_proj_prefix()
            attention(hp, qa, ka, ng)
            if ng is not None:
                for _ in ng:  # drain leftovers
                    pass
            qa, ka = nqa, nka
        while norm_pending:
            flush_norm(norm_pending[:2])
            del norm_pending[:2]

        # ---- output projection (transposed) ----
        # NOTE: must be emitted entirely AFTER the attention loop: Tile
        # dependencies follow emission order, so reads of zt must come after
        # all writes.
        # group order: ct0-qn0 finisher first, ct0-qn1 finisher deferred past
        # ct1 (it waits on hp7-qn1's normalize chain) and emitted LAST so the
        # tail chain is a single short matmul+evac+DMA.
        def emit_group(ct, qn, split=False):
            pw_t = pw_sb[:, ct * 1024: (ct + 1) * 1024]
            if ct == 0:
                # finish the prefix accumulation started during hp7
                ps = ps_pre[qn]
                nc.tensor.matmul(
                    ps, pw_t[:, (DT - 1) * 128: DT * 128],
                    zt[DT - 1][:, qn * 512: qn * 512 + 512], start=False, stop=True,
                )
            else:
                ps = psum.tile([128, 512], f32, tag="big", name="ps_o", bufs=2)
                for jt in range(DT):
                    nc.tensor.matmul(
                        ps,
                        pw_t[:, jt * 128: jt * 128 + 128],
                        zt[jt][:, qn * 512: qn * 512 + 512],
                        start=(jt == 0), stop=(jt == DT - 1),
                    )
            # split=True: halve the final evac+DMA chain on the critical tail
            for off, wid in ([(0, 256), (256, 256)] if split else [(0, 512)]):
                ot = mpool.tile([128, wid], f32, tag="ot", name="ot", bufs=4)
                nc.scalar.activation(ot, ps[:, off: off + wid],
                                     AF.Identity, bias=pb_sb[:, ct: ct + 1])
                nc.sync.dma_start(
                    outT[ct * 128: ct * 128 + 128,
                         qn * 512 + off: qn * 512 + off + wid], ot)

        emit_group(0, 0)
        for ct in range(1, DT):
            emit_group(ct, 0)
            emit_group(ct, 1)
        emit_group(0, 1, split=True)

    nc.compile()
    return nc


def prep_inputs(x, qkv_w, qkv_b, proj_w, proj_b):
    x = np.asarray(x, dtype=np.float32)
    qkv_w = np.asarray(qkv_w, dtype=np.float32)
    qkv_b = np.asarray(qkv_b, dtype=np.float32)
    proj_w = np.asarray(proj_w, dtype=np.float32)
    proj_b = np.asarray(proj_b, dtype=np.float32)

    # x^T packed: [b, p, kt*N + n] = x[b, n, 128kt+p]
    xp = (x.transpose(0, 2, 1).reshape(NB, DT, 128, N).transpose(0, 2, 1, 3)
          .reshape(NB, 128, DT * N).astype(np.float16))

    wqkT = qkv_w[:2048, :].T                                  # [d, j']
    A4 = wqkT.reshape(DT, 128, 16, 128).transpose(1, 0, 2, 3)  # [p, kt, jt, jj]
    wqk_packed = (np.stack([A4[:, :, 0:8, :], A4[:, :, 8:16, :]], axis=3)
                  .transpose(0, 2, 1, 3, 4).reshape(128, HP * 2048).astype(np.float16))

    wvT = qkv_w[2048:, :].T                                   # [d, j]
    wv_packed = (wvT.reshape(DT, 128, 1024).transpose(1, 0, 2)
                 .reshape(128, 8192).astype(np.float16))

    pwT = proj_w.T                                            # [j, c]
    pw_packed = (pwT.reshape(DT, 128, DT, 128).transpose(1, 2, 0, 3)
                 .reshape(128, 8192).astype(np.float16))

    bqk_pt = np.ascontiguousarray(qkv_b[:2048].reshape(16, 128).T)
    bv_r = qkv_b[2048:].reshape(1, 1024).astype(np.float16)
    pb_pt = np.ascontiguousarray(proj_b.reshape(8, 128).T)

    shared = {
        "wqk": wqk_packed, "wv": wv_packed, "pw": pw_packed,
        "bqk": bqk_pt, "bv": bv_r, "pb": pb_pt,
    }
    return [{**shared, "xp": xp[b]} for b in range(NB)]


def kernel(x, qkv_w, qkv_b, proj_w, proj_b):
    global LAST_RESULTS, _NC_CACHE
    if _NC_CACHE is None:
        _NC_CACHE = build_nc()
    nc = _NC_CACHE
    in_maps = prep_inputs(x, qkv_w, qkv_b, proj_w, proj_b)
    res = run_bass_kernel_spmd(
        nc, in_maps, core_ids=list(range(NB)),
        trace=bool(os.environ.get("BASS_TRACE")),
    )
    LAST_RESULTS = res
    out = np.stack([np.ascontiguousarray(res.results[b]["outT"].T) for b in range(NB)])
    return out


# revision 75
# speedup vs baseline: 2.0502x; 1.1143x over previous
"""Multi-head attention block (B=8, N=1024, D=1024, H=16, dh=64) on 8 TRN2 NeuronCores.

Strategy: data-parallel over batch (1 batch element per core). Per core, the whole
attention block runs out of SBUF in a feature-major ("transposed") dataflow that
avoids all on-device transposes:

  - qT/kT computed feature-major:  qkT[j, n]  = sum_d qkv_w[j, d] * x[n, d]   (lhsT=Wqk^T, rhs=x^T)
  - v computed token-major:        v[n, j]    = sum_d x[n, d] * Wv[j, d]      (lhsT=x^T, rhs=Wv^T)
  - scores transposed:             sT[k, q]   = sum_dh kT[dh, k] * qT[dh, q]  (K=64, row-packed head pairs)
  - pattern:                       pT = exp(SCALE * sT)                        (ACT, PSUM->SBUF, fp16)
  - zT + denominator fused:        [zT_h; den] = [v_h | 1]^T @ pT              (M=65, ones col)
  - normalize:                     zT_h *= bcast(1/den)   (DVE recip -> GPSIMD partition_broadcast -> DVE mul)
  - output transposed:             outT[c, q] = sum_j proj_w[c, j] zT[j, q] + pb[c]

All matmul operands live in SBUF as fp16 (same 1 cyc/row PE rate as bf16/f32r
with free>=256, but half the DMA/SBUF footprint); accumulation is fp32 PSUM.

Pipeline structure (all engines near-continuously busy):
  - V projection runs kt-outer in 2-token-tile waves over 4 rotating PSUM
    slot-pairs, so the first matmul needs only the first x/wv DMA chunks and
    waves never wait on evacuations; the hp0 qk projection interleaves into
    the last wave.
  - Per head-pair, the next pair's qk projection interleaves into the
    attention kt loop; the last pair interleaves the first output-projection
    partials instead.
  - Softmax normalization is deferred off the critical path: z/den rows are
    copied to SBUF right away (freeing PSUM), and each head-pair's
    recip/broadcast/multiply burst is flushed at the START of the next
    pair's attention (one GPSIMD broadcast launch per head-pair). This is
    worth ~100us/iter on HW: the ucode launch + cross-engine semaphore chain
    otherwise gates the PSUM rotation at every qn boundary.
  - proj weights prefetch during attention; the ct0 groups finish partials
    started during hp7, with the qn1 finisher emitted last (it waits on
    hp7's normalize) and split for a short tail.
"""
import os
import numpy as np
from contextlib import ExitStack

import concourse.bacc as bacc
import concourse.tile as tile
from concourse import mybir
from concourse.bass_utils import run_bass_kernel_spmd

f32 = mybir.dt.float32
f32r = mybir.dt.float32r
f16 = mybir.dt.float16
AF = mybir.ActivationFunctionType

NB = 8          # batch / cores
N = 1024        # tokens
D = 1024        # d_model
H = 16          # heads
DH = 64         # head dim
SCALE = DH ** -0.5
NT = N // 128   # 8 token tiles
DT = D // 128   # 8 d tiles
HP = H // 2     # 8 head pairs

# Stashed results of the last run (for test harness introspection)
LAST_RESULTS = None
_NC_CACHE = None


def build_nc(loop_r=None):
    nc = bacc.Bacc("TRN2", target_bir_lowering=False, debug=False, enable_asserts=False)

    xp = nc.dram_tensor("xp", [128, DT * N], f16, kind="ExternalInput").ap()
    wqk = nc.dram_tensor("wqk", [128, HP * 2048], f16, kind="ExternalInput").ap()
    wv = nc.dram_tensor("wv", [128, 8192], f16, kind="ExternalInput").ap()
    pw = nc.dram_tensor("pw", [128, 8192], f16, kind="ExternalInput").ap()
    bqk = nc.dram_tensor("bqk", [128, 16], f32, kind="ExternalInput").ap()
    bv = nc.dram_tensor("bv", [1, 1024], f16, kind="ExternalInput").ap()
    pb = nc.dram_tensor("pb", [128, 8], f32, kind="ExternalInput").ap()
    outT = nc.dram_tensor("outT", [D, N], f32, kind="ExternalOutput").ap()

    with tile.TileContext(nc) as tc, ExitStack() as ctx:
        const = ctx.enter_context(tc.tile_pool(name="const", bufs=1))
        xpool = ctx.enter_context(tc.tile_pool(name="xp", bufs=1))
        wvpool = ctx.enter_context(tc.tile_pool(name="wvp", bufs=1))
        vpool = ctx.enter_context(tc.tile_pool(name="vp", bufs=1))
        qkpool = ctx.enter_context(tc.tile_pool(name="qkp", bufs=4))
        ztpool = ctx.enter_context(tc.tile_pool(name="ztp", bufs=1))
        wqkpool = ctx.enter_context(tc.tile_pool(name="wqkp", bufs=2))
        pwpool = ctx.enter_context(tc.tile_pool(name="pwp", bufs=1))
        ptpool = ctx.enter_context(tc.tile_pool(name="ptp", bufs=4))
        mpool = ctx.enter_context(tc.tile_pool(name="mp", bufs=2))
        psum = ctx.enter_context(tc.tile_pool(name="ps", bufs=1, space="PSUM"))

        if loop_r is not None:
            ctx.enter_context(tc.For_i(
                0, loop_r, 1,
                hint_engines=(mybir.EngineType.PE, mybir.EngineType.Activation,
                              mybir.EngineType.DVE, mybir.EngineType.SP,
                              mybir.EngineType.Pool),
            ))

        # ---- persistent activations. Three HWDGE queues run in parallel:
        # wv chunks on ACT (idle until attention), xT pieces on SP (strided,
        # one issue per piece class; V runs kt-outer so wave w only needs
        # xT cols [256w, 256w+256) of each chunk), consts on DVE. ----
        bv_row = const.tile([1, 1024], f16, tag="bvr")
        bvrep = const.tile([128, 1024], f16, tag="bvrep")

        xT = xpool.tile([128, DT * N], f16, tag="xT")        # [p, kt*N + n] = x[n, 128kt+p]
        wv_sb = wvpool.tile([128, 8192], f16, tag="wv")      # [p, kt*1024 + j]
        x3 = xT.rearrange("p (kt n) -> p kt n", n=N)
        s3 = xp.rearrange("p (kt n) -> p kt n", n=N)
        nc.scalar.dma_start(wv_sb[:, 0:1024], wv[:, 0:1024])
        for g4 in range(4):
            if g4 > 0:
                nc.scalar.dma_start(wv_sb[:, g4 * 2048: (g4 + 1) * 2048],
                                    wv[:, g4 * 2048: (g4 + 1) * 2048])
            nc.sync.dma_start(xT[:, 2 * g4 * N: (2 * g4 + 2) * N],
                              xp[:, 2 * g4 * N: (2 * g4 + 2) * N])
            if g4 == 0:
                nc.scalar.dma_start(wv_sb[:, 1024:2048], wv[:, 1024:2048])
            if g4 == 1:
                nc.sync.dma_start(bv_row, bv)
                nc.gpsimd.partition_broadcast(bvrep, bv_row)

        bqk_sb = const.tile([128, 16], f32, tag="bqk")
        nc.scalar.dma_start(bqk_sb, bqk)
        pb_sb = const.tile([128, 8], f32, tag="pb")
        nc.scalar.dma_start(pb_sb, pb)

        v_sb = []                                            # [p=token, 65h + c]; col 65h+64 == 1.0
        for tt in range(NT):
            vt = vpool.tile([128, H * 65], f16, tag=f"v{tt}", name=f"v{tt}")
            nc.gpsimd.memset(vt.rearrange("p (h c) -> p h c", c=65)[:, :, 64], 1.0)
            v_sb.append(vt)

        zt = []                                              # [p=feature within tile, q]
        for jt in range(DT):
            zt.append(ztpool.tile([128, N], f16, tag=f"z{jt}", name=f"z{jt}"))

        # ---- interleaved qkT projection + attention ----
        def make_qk_proj(hp):
            """Returns (qa, ka, generator). Generator emits 2 PE matmuls per step,
            16 steps total, with the DVE bias-evacuation attached to group ends."""
            wqk_t = wqkpool.tile([128, 2048], f16, tag="wqk", name=f"wqk{hp}")
            nc.sync.dma_start(wqk_t, wqk[:, hp * 2048: (hp + 1) * 2048])
            qa = qkpool.tile([128, N], f16, tag="qk", name=f"qa{hp}")
            ka = qkpool.tile([128, N], f16, tag="qk", name=f"ka{hp}")

            def gen():
                for dest, jt, which in ((qa, hp, 0), (ka, 8 + hp, 1)):
                    for qn in range(2):
                        ps = psum.tile([128, 512], f32, tag="mm", name="ps_qk", bufs=1)
                        for kt in range(DT):
                            base = kt * 256 + which * 128
                            nc.tensor.matmul(
                                ps,
                                wqk_t[:, base: base + 128],
                                xT[:, kt * N + qn * 512: kt * N + qn * 512 + 512],
                                start=(kt == 0), stop=(kt == DT - 1),
                            )
                            if kt % 2 == 1:
                                if kt == DT - 1:
                                    with nc.allow_low_precision(reason="f16 qk evac"):
                                        nc.vector.tensor_scalar_add(
                                            dest[:, qn * 512: qn * 512 + 512], ps,
                                            bqk_sb[:, jt: jt + 1],
                                        )
                                yield
            return qa, ka, gen()

        # ---- phase V: v projection, kt-outer 2-tile waves. PSUM slot-pairs
        # rotate over A,B (big), C (zps0+zps1), D (zps2+mm) so a wave never
        # waits on the previous wave's evacuation; the hp0 qk projection is
        # interleaved into the last wave out of the then-free mm slot. ----
        qa, ka, g = make_qk_proj(0)

        def v_slot(kind):
            if kind == "A" or kind == "B":
                t = psum.tile([128, 1024], f32, tag="big", name=f"vps{kind}", bufs=2)
                return [t[:, 0:512], t[:, 512:1024]]
            if kind == "C":
                return [psum.tile([128, 512], f32, tag="zps", name="vpsc0", bufs=3),
                        psum.tile([128, 512], f32, tag="zps", name="vpsc1", bufs=3)]
            return [psum.tile([128, 512], f32, tag="zps", name="vpsd0", bufs=3),
                    psum.tile([128, 512], f32, tag="mm", name="vpsd1", bufs=1)]

        wave_slots = [("A", "B"), ("C", "D"), ("A", "B"), ("C", "A")]
        for w in range(NT // 2):
            vps = [v_slot(k) for k in wave_slots[w]]
            for kt in range(DT):
                for i in range(2):
                    tt = 2 * w + i
                    for jn in range(2):
                        nc.tensor.matmul(
                            vps[i][jn],
                            xT[:, kt * N + tt * 128: kt * N + tt * 128 + 128],
                            wv_sb[:, kt * 1024 + jn * 512: kt * 1024 + jn * 512 + 512],
                            start=(kt == 0), stop=(kt == DT - 1),
                        )
                if w == NT // 2 - 1 and kt >= 2:
                    next(g, None)
                    next(g, None)
                    next(g, None)
            for i in range(2):
                tt = 2 * w + i
                v3 = v_sb[tt].rearrange("p (h c) -> p h c", c=65)
                b3 = bvrep.rearrange("p (h c) -> p h c", c=64)
                for jn in range(2):
                    with nc.allow_low_precision(reason="f16 v evac"):
                        nc.vector.tensor_add(
                            v3[:, jn * 8: jn * 8 + 8, 0:64],
                            vps[i][jn].rearrange("p (h c) -> p h c", c=64),
                            b3[:, jn * 8: jn * 8 + 8, :],
                        )
        for _ in g:  # drain leftovers
            pass

        # whole proj weight, preloaded during attention (see hp loop)
        pw_sb = pwpool.tile([128, 8192], f16, tag="pw")

        def make_proj_prefix():
            """Filler for the last head-pair: proj (ct0, qn0/qn1) partials
            jt 0..6 accumulate in the freed mm / third zps slots while hp7's
            attention runs."""
            ps_pre = [psum.tile([128, 512], f32, tag="mm", name="ps_o0", bufs=1), None]

            def gen():
                # jt<=5 first: zt[6]'s deferred muls (hp6 flush phase 2) are
                # emitted after qn0's copies, so both jt==6 partials must wait
                # for the qn1 steps (>=9) of this generator
                for jt in range(DT - 2):
                    nc.tensor.matmul(
                        ps_pre[0], pw_sb[:, jt * 128: jt * 128 + 128],
                        zt[jt][:, 0:512], start=(jt == 0), stop=False,
                    )
                    yield
                # allocated mid-qn0 so the zps rotation skips hp7's live slots
                ps_pre[1] = psum.tile([128, 512], f32, tag="zps", name="ps_o1", bufs=3)
                for jt in range(2):
                    nc.tensor.matmul(
                        ps_pre[1], pw_sb[:, jt * 128: jt * 128 + 128],
                        zt[jt][:, 512:1024], start=(jt == 0), stop=False,
                    )
                    yield
                nc.tensor.matmul(
                    ps_pre[0], pw_sb[:, (DT - 2) * 128: (DT - 1) * 128],
                    zt[DT - 2][:, 0:512], start=False, stop=False,
                )
                yield
                for jt in range(2, DT - 1):
                    nc.tensor.matmul(
                        ps_pre[1], pw_sb[:, jt * 128: jt * 128 + 128],
                        zt[jt][:, 512:1024], start=(jt == 0), stop=False,
                    )
                    yield
            return ps_pre, gen()

        # deferred softmax-normalize: both qn of a head-pair flushed in one
        # burst (one broadcast ucode launch per hp) after the NEXT head-pair's
        # first z-evacuation, keeping the DVE queue clear at qn boundaries
        norm_pending = []
        norm_staged = []  # (entries, bc) whose muls are deferred (two-phase)

        def emit_norm_muls(entries, bc):
            for i, (hp, qn, zsb) in enumerate(entries):
                for h in range(2):
                    with nc.allow_low_precision(reason="f16 attn out"):
                        nc.vector.tensor_mul(
                            zt[hp][h * 64: h * 64 + 64, qn * 512: qn * 512 + 512],
                            zsb[h][0:64, :],
                            bc[:, (2 * i + h) * 512: (2 * i + h) * 512 + 512])

        def flush_norm(entries, two_phase=False):
            wid = len(entries) * 1024
            recip = mpool.tile([1, wid], f32r, tag="recip", name="recip", bufs=2)
            for i, (hp, qn, zsb) in enumerate(entries):
                for h in range(2):
                    with nc.allow_low_precision(reason="f32r rounding of softmax denom"):
                        nc.vector.reciprocal(
                            recip[:, (2 * i + h) * 512: (2 * i + h) * 512 + 512],
                            zsb[h][64:65, :])
            bc = mpool.tile([64, wid], f32r, tag="bc", name="bc", bufs=2)
            nc.gpsimd.partition_broadcast(bc, recip)
            if two_phase:
                # muls wait on the broadcast; emitted later (once it has
                # completed) so they never block the in-order DVE queue
                norm_staged.append((list(entries), bc))
            else:
                emit_norm_muls(entries, bc)

        def attention(hp, qa, ka, filler):
            # flush the previous head-pair's deferred normalize first; the
            # hp7 prefix filler is ordered so its zt[6] reads emit after the
            # staged muls land (mid-qn0), keeping even hp6's flush two-phase
            if norm_pending:
                flush_norm(norm_pending[:], two_phase=True)
                del norm_pending[:]

            def emit_sps_exp(qn, kt):
                sps = psum.tile([128, 1024], f32, tag="big", name="sps", bufs=2)
                for h in range(2):
                    off = h * 64
                    nc.tensor.matmul(
                        sps[:, h * 512: h * 512 + 512],
                        ka[off: off + 64, kt * 128: kt * 128 + 128],
                        qa[off: off + 64, qn * 512: qn * 512 + 512],
                        start=True, stop=True,
                    )
                pt = ptpool.tile([128, 1024], f16, tag="pt", name="pt")
                nc.scalar.activation(pt, sps, AF.Exp, scale=SCALE)
                return pt

            for qn in range(2):
                if hp == HP - 1 and qn == 1 and norm_pending:
                    # last head-pair: flush qn0's normalize now, hidden under
                    # qn1's compute (DVE/Pool are idle here — the hp7 filler
                    # is PE-only), so proj doesn't wait on it
                    flush_norm(norm_pending[:], two_phase=True)
                    del norm_pending[:]
                zps = [psum.tile([65, 512], f32, tag="zps", name=f"zps{h}", bufs=3)
                       for h in range(2)]
                pt_next = emit_sps_exp(qn, 0)
                for kt in range(NT):
                    pt = pt_next
                    if kt + 1 < NT:
                        pt_next = emit_sps_exp(qn, kt + 1)
                    if filler is not None:
                        next(filler, None)
                    for h in range(2):
                        nc.tensor.matmul(
                            zps[h],
                            v_sb[kt][:, 65 * (2 * hp + h): 65 * (2 * hp + h) + 65],
                            pt[:, h * 512: h * 512 + 512],
                            start=(kt == 0), stop=(kt == NT - 1),
                        )
                # evacuate zps to SBUF immediately (frees the PSUM slot for
                # the next qn); the actual normalize runs later, off the
                # critical path (see flush_norm)
                zsb = [mpool.tile([65, 512], f32r, tag="zsb", name=f"zsb{h}", bufs=8)
                       for h in range(2)]
                for h in range(2):
                    with nc.allow_low_precision(reason="f32r z evac"):
                        nc.vector.tensor_copy(zsb[h], zps[h])
                norm_pending.append((hp, qn, zsb))
                if qn == 0 and norm_staged:
                    # the staged muls' broadcast finished long ago: emit them
                    # now without blocking the DVE queue
                    for entries, bc in norm_staged:
                        emit_norm_muls(entries, bc)
                    del norm_staged[:]

        ps_pre = None
        for hp in range(HP):
            if hp == 4:
                # prefetch proj weights while the DMA queues are idle
                nc.sync.dma_start(pw_sb[:, 0:4096], pw[:, 0:4096])
                nc.sync.dma_start(pw_sb[:, 4096:8192], pw[:, 4096:8192])
            if hp + 1 < HP:
                nqa, nka, ng = make_qk_proj(hp + 1)
            else:
                nqa = nka = None
                ps_pre, ng = make_proj_prefix()
            attention(hp, qa, ka, ng)
            if ng is not None:
                for _ in ng:  # drain leftovers
                    pass
            qa, ka = nqa, nka
        for entries, bc in norm_staged:  # drain hp7-qn0's staged muls
            emit_norm_muls(entries, bc)
        del norm_staged[:]
        while norm_pending:  # hp7-qn1: muls deferred into early proj emission
            flush_norm(norm_pending[:2], two_phase=True)
            del norm_pending[:2]

        # ---- output projection (transposed) ----
        # NOTE: must be emitted entirely AFTER the attention loop: Tile
        # dependencies follow emission order, so reads of zt must come after
        # all writes.
        # group order: ct0-qn0 finisher first, ct0-qn1 finisher deferred past
        # ct1 (it waits on hp7-qn1's normalize chain) and emitted LAST so the
        # tail chain is a single short matmul+evac+DMA.
        def emit_group(ct, qn, evac_dve=False):
            pw_t = pw_sb[:, ct * 1024: (ct + 1) * 1024]
            if ct == 0:
                # finish the prefix accumulation started during hp7
                ps = ps_pre[qn]
                nc.tensor.matmul(
                    ps, pw_t[:, (DT - 1) * 128: DT * 128],
                    zt[DT - 1][:, qn * 512: qn * 512 + 512], start=False, stop=True,
                )
            else:
                ps = psum.tile([128, 512], f32, tag="big", name="ps_o", bufs=2)
                for jt in range(DT):
                    nc.tensor.matmul(
                        ps,
                        pw_t[:, jt * 128: jt * 128 + 128],
                        zt[jt][:, qn * 512: qn * 512 + 512],
                        start=(jt == 0), stop=(jt == DT - 1),
                    )
            # evac_dve: use DVE (idle at the tail) so the last chains run in
            # parallel with ACT's
            ot = mpool.tile([128, 512], f32, tag="ot", name="ot", bufs=4)
            if evac_dve:
                nc.vector.tensor_scalar_add(ot, ps, pb_sb[:, ct: ct + 1])
            else:
                nc.scalar.activation(ot, ps, AF.Identity, bias=pb_sb[:, ct: ct + 1])
            nc.sync.dma_start(
                outT[ct * 128: ct * 128 + 128, qn * 512: qn * 512 + 512], ot)

        emit_group(0, 0)
        emit_group(1, 0)
        # hp7-qn1's staged muls: their broadcast completed during the first
        # two groups, so they slot into the DVE queue without blocking; they
        # must precede the first zt[7] qn1 reader (ct1, qn1)
        for entries, bc in norm_staged:
            emit_norm_muls(entries, bc)
        del norm_staged[:]
        emit_group(1, 1)
        for ct in range(2, DT):
            emit_group(ct, 0, evac_dve=(ct == DT - 1))
            emit_group(ct, 1)
        emit_group(0, 1, evac_dve=True)

    nc.compile()
    return nc


def prep_inputs(x, qkv_w, qkv_b, proj_w, proj_b):
    x = np.asarray(x, dtype=np.float32)
    qkv_w = np.asarray(qkv_w, dtype=np.float32)
    qkv_b = np.asarray(qkv_b, dtype=np.float32)
    proj_w = np.asarray(proj_w, dtype=np.float32)
    proj_b = np.asarray(proj_b, dtype=np.float32)

    # x^T packed: [b, p, kt*N + n] = x[b, n, 128kt+p]
    xp = (x.transpose(0, 2, 1).reshape(NB, DT, 128, N).transpose(0, 2, 1, 3)
          .reshape(NB, 128, DT * N).astype(np.float16))

    wqkT = qkv_w[:2048, :].T                                  # [d, j']
    A4 = wqkT.reshape(DT, 128, 16, 128).transpose(1, 0, 2, 3)  # [p, kt, jt, jj]
    wqk_packed = (np.stack([A4[:, :, 0:8, :], A4[:, :, 8:16, :]], axis=3)
                  .transpose(0, 2, 1, 3, 4).reshape(128, HP * 2048).astype(np.float16))

    wvT = qkv_w[2048:, :].T                                   # [d, j]
    wv_packed = (wvT.reshape(DT, 128, 1024).transpose(1, 0, 2)
                 .reshape(128, 8192).astype(np.float16))

    pwT = proj_w.T                                            # [j, c]
    pw_packed = (pwT.reshape(DT, 128, DT, 128).transpose(1, 2, 0, 3)
                 .reshape(128, 8192).astype(np.float16))

    bqk_pt = np.ascontiguousarray(qkv_b[:2048].reshape(16, 128).T)
    bv_r = qkv_b[2048:].reshape(1, 1024).astype(np.float16)
    pb_pt = np.ascontiguousarray(proj_b.reshape(8, 128).T)

    shared = {
        "wqk": wqk_packed, "wv": wv_packed, "pw": pw_packed,
        "bqk": bqk_pt, "bv": bv_r, "pb": pb_pt,
    }
    return [{**shared, "xp": xp[b]} for b in range(NB)]


def kernel(x, qkv_w, qkv_b, proj_w, proj_b):
    global LAST_RESULTS, _NC_CACHE
    if _NC_CACHE is None:
        _NC_CACHE = build_nc()
    nc = _NC_CACHE
    in_maps = prep_inputs(x, qkv_w, qkv_b, proj_w, proj_b)
    res = run_bass_kernel_spmd(
        nc, in_maps, core_ids=list(range(NB)),
        trace=bool(os.environ.get("BASS_TRACE")),
    )
    LAST_RESULTS = res
    out = np.stack([np.ascontiguousarray(res.results[b]["outT"].T) for b in range(NB)])
    return out
